# revision 2
# baseline (speedup 1.0000x reference)
"""Bipartite GNN edge decoder on 8 Trainium2 NeuronCores.

Per edge e: out = sigmoid(w2 . relu(W1a z_src[row] + W1b z_dst[col] + b1) + b2)

Distribution: 4 src-windows x 2 dst-windows of 25000 rows; each core owns
one (src-win, dst-win) pair (~125k edges).

Per core:
  - u table (z_src@W1a.T + b1, bf16) built on device into scratch DRAM
    in 1536-row chunks sharing the edge phase's PSUM shape; one batched
    DVE bias-add+cast per chunk.
  - Edges sorted by col, packed into 512-edge groups with <=128 distinct
    cols; the host relabels each group's cols to contiguous slots and
    uploads a permuted/duplicated z_dst table so group g's v rows are
    exactly SBUF panel g (fully static schedule).
  - Edge phase in super-groups of 3 groups = 1536 edges = one SWDGE
    dma_gather (12 chunks of 128 edges), [edge, feat] layout:
      host-uploaded one-hot S [slot, 1536] bf16 (streamed over HWDGE)
      3 identity-matmuls accumulate the gathered u tile into PSUM,
      12 S-block-stationary matmuls add the v panels (PSUM accumulate)
      ACT relu PSUM->SBUF; |w2| is folded into the tables host-side with
      features permuted so w2>0 comes first: logit = reduce(r[:, :, :k])
      - reduce(r[:, :, k:]) (two DVE tensor_reduce + a subtract)
      ACT sigmoid on [128, 12]; 12KB stores every 2 super-groups.
  - Gathers round-robin all 4 SWDGE queues; each queue's ring drains at
    ~32GB/s (256B descriptors), 4-way concurrent => ~2ns/edge, which is
    the kernel's floor.

Measured: 984518 ns (prior session's kernel) -> ~398000 ns (2.47x).

The host inverse-permutes the outputs.
"""
import os
import numpy as np

import concourse.bass as bass
import concourse.bacc as bacc
import concourse.mybir as mybir
from concourse.tile import TileContext
from concourse.bass_utils import run_bass_kernel_spmd

N_SRC, N_DST, E, H = 100000, 50000, 1000000, 128
N_CORES = 8
P = 128

WIN_SRC = N_SRC // 4        # 25000
WIN_DST = N_DST // 2        # 25000
GE = 512                    # edges per group
SG = 3                      # groups per super-group (= one gather)
BG = SG * GE                # 1536 idx per gather
NCH = BG // P               # 12
SB = 6                      # groups per output store block (2 supers)
CH = 1536                   # u-table build chunk (rows)
KT = CH // P                # 12
N_UCHUNK = -(-WIN_SRC // CH)    # 17
UTAB_ROWS = N_UCHUNK * CH       # 26112
CG = 6                      # groups per colrep chunk (2 supers)

PRE_G = 10                  # gather prefetch depth (super-groups)
PV = 8                      # v-panel build lookahead (groups)
PC = 2                      # colrep chunk prefetch

_cache = {}
_last_results = None


def _uslot(r):
    """z row id -> u-table slot under the chunk store layout."""
    c, w = np.divmod(r.astype(np.int64), CH)
    k, p = np.divmod(w, P)
    return c * CH + p * KT + k


def _wrap_idx_all(slots, ngath):
    """[ngath*BG] int -> [128, ngath*BG//16] int16 SWDGE idx layout."""
    arr = slots.reshape(ngath, BG // 16, 16).transpose(0, 2, 1)  # [g,16,96]
    rep = np.tile(arr, (1, 8, 1))                                # [g,128,96]
    out = rep.transpose(1, 0, 2).reshape(P, ngath * (BG // 16))
    return np.ascontiguousarray(out.astype(np.int16))


def _build_program(ncap, ksplit):
    fp32 = mybir.dt.float32
    bf16 = mybir.dt.bfloat16
    i16 = mybir.dt.int16
    i8 = mybir.dt.int8
    nsup = ncap // SG
    ngath = nsup
    ecap = ncap * GE
    nblk = ncap // SB
    nc = bacc.Bacc(trn_type="TRN2", num_swdge_queues=4)

    zsT_d = nc.dram_tensor("zsT", [H, UTAB_ROWS], bf16, kind="ExternalInput")
    zdT_d = nc.dram_tensor("zdT", [H, ncap * P], bf16, kind="ExternalInput")
    ix_d = nc.dram_tensor("ix", [P, ngath * (BG // 16)], i16,
                          kind="ExternalInput")
    S_d = nc.dram_tensor("S", [P, ecap], bf16, kind="ExternalInput")
    w1aT_d = nc.dram_tensor("w1aT", [H, H], fp32, kind="ExternalInput")
    w1bT_d = nc.dram_tensor("w1bT", [H, H], fp32, kind="ExternalInput")
    b1rep_d = nc.dram_tensor("b1rep", [P, H], fp32, kind="ExternalInput")
    b2rep_d = nc.dram_tensor("b2rep", [P, 1], fp32, kind="ExternalInput")
    out_d = nc.dram_tensor("out", [nblk, P, SB * 4], fp32,
                           kind="ExternalOutput")

    with TileContext(nc) as tc:
        with (
            tc.tile_pool(name="const", bufs=1) as cpool,
            tc.tile_pool(name="dram", bufs=1, space="DRAM") as dpool,
            tc.tile_pool(name="work", bufs=3) as wpool,
            tc.tile_pool(name="psum", bufs=2, space="PSUM") as ppool,
        ):
            # ---- critical-path consts only (rest loaded during phase A) ----
            w1aT_f = cpool.tile([H, H], fp32)
            nc.sync.dma_start(out=w1aT_f[:], in_=w1aT_d[:])
            w1aT = cpool.tile([H, H], bf16)
            nc.vector.tensor_copy(w1aT[:], w1aT_f[:])
            b1rep = cpool.tile([P, H], fp32)
            nc.sync.dma_start(out=b1rep[:], in_=b1rep_d[:])

            # ---- deferred const tiles ----
            from concourse.masks import make_identity
            w1bT_f = cpool.tile([H, H], fp32)
            w1bT = cpool.tile([H, H], bf16)
            identb = cpool.tile([P, P], bf16)
            b2rep = cpool.tile([P, 1], fp32)
            ixs = cpool.tile([P, ngath * (BG // 16)], i16)

            # ---- v-panel builder (batched per super-group) ----
            vtab = cpool.tile([P, ncap, H], bf16)
            vb_next = [0]

            def emit_vbuild_super():
                s0 = vb_next[0]
                if s0 >= nsup:
                    return
                vb_next[0] += 1
                zT = wpool.tile([H, SG * P], bf16, tag="zT", bufs=4)
                nc.sync.dma_start(out=zT[:],
                                  in_=zdT_d[:, s0 * SG * P:(s0 + 1) * SG * P])
                ps = ppool.tile([P, SG, H], mybir.dt.float32, tag="vb")
                for gg in range(SG):
                    nc.tensor.matmul(ps[:, gg, :],
                                     lhsT=zT[:, gg * P:(gg + 1) * P],
                                     rhs=w1bT[:], start=True, stop=True)
                nc.vector.tensor_tensor(
                    vtab[:, s0 * SG:(s0 + 1) * SG, :], ps[:],
                    b1rep[:, None, :].to_broadcast([P, SG, H]),
                    op=mybir.AluOpType.add)

            # ---- phase A: u table (bf16, scratch DRAM), v-builds woven in ----
            utab = dpool.tile([UTAB_ROWS, H], bf16)
            for c in range(N_UCHUNK):
                zs = wpool.tile([H, CH], bf16, tag="zs", bufs=4)
                nc.sync.dma_start(out=zs[:], in_=zsT_d[:, c * CH:(c + 1) * CH])
                ps = ppool.tile([P, KT, H], mybir.dt.float32, tag="t")
                for k in range(KT):
                    nc.tensor.matmul(ps[:, k, :],
                                     lhsT=zs[:, k * P:(k + 1) * P],
                                     rhs=w1aT[:], start=True, stop=True)
                us = wpool.tile([P, KT, H], bf16, tag="us", bufs=3)
                nc.scalar.copy(us[:], ps[:])
                nc.sync.dma_start(
                    out=utab[c * CH:(c + 1) * CH]
                        .rearrange("(p k) f -> p k f", p=P),
                    in_=us[:])
                if c == 0:
                    # weave the remaining const loads behind chunk 0
                    nc.sync.dma_start(out=ixs[:], in_=ix_d[:])
                    nc.sync.dma_start(out=w1bT_f[:], in_=w1bT_d[:])
                    nc.vector.tensor_copy(w1bT[:], w1bT_f[:])
                    make_identity(nc, identb[:])
                    nc.sync.dma_start(out=b2rep[:], in_=b2rep_d[:])

            # ---- phase B: edge super-groups ----
            for _ in range(4):
                emit_vbuild_super()

            gtiles = {}

            def emit_gather(j):
                ug = wpool.tile([P, NCH, H], bf16, tag="ug", bufs=PRE_G + 2)
                nc.gpsimd.dma_gather(
                    out_ap=ug[:], in_ap=utab[:],
                    idxs_ap=ixs[:, j * (BG // 16):(j + 1) * (BG // 16)],
                    num_idxs=BG, num_idxs_reg=BG, elem_size=H,
                    queue_num=j % 4, single_packet=False)
                gtiles[j] = ug

            stiles = {}

            def emit_sload(i):
                # one DMA covers supers 2i and 2i+1
                St = wpool.tile([P, 2 * BG], bf16, tag="S", bufs=PC + 2)
                lo = i * 2 * BG
                hi = min((i + 1) * 2 * BG, ecap)
                nc.sync.dma_start(out=St[:, :hi - lo], in_=S_d[:, lo:hi])
                stiles[i] = St

            for j in range(min(PRE_G, ngath)):
                emit_gather(j)
            for i in range(min(PC + 1, -(-nsup // 2))):
                emit_sload(i)

            coll = wpool.tile([P, SB * 4], fp32, tag="coll", bufs=2)
            for sg in range(nsup):
                g0 = sg * SG
                if sg + PRE_G < ngath:
                    emit_gather(sg + PRE_G)
                emit_vbuild_super()
                if sg % 2 == 0 and sg // 2 + PC + 1 < -(-nsup // 2):
                    emit_sload(sg // 2 + PC + 1)
                Sw = stiles[sg // 2]
                sb0 = (sg % 2) * BG
                if sg % 2 == 1 or sg == nsup - 1:
                    stiles.pop(sg // 2, None)

                t = ppool.tile([P, NCH, H], mybir.dt.float32, tag="t")
                ug = gtiles.pop(sg)
                for gg in range(SG):
                    nc.tensor.matmul(
                        t[:, gg * 4:(gg + 1) * 4, :], lhsT=identb[:],
                        rhs=ug[:, gg * 4:(gg + 1) * 4, :],
                        start=True, stop=False)
                for cc in range(NCH):
                    nc.tensor.matmul(t[:, cc, :],
                                     lhsT=Sw[:, sb0 + cc * P:sb0 + (cc + 1) * P],
                                     rhs=vtab[:, g0 + cc // 4, :],
                                     start=False, stop=True)
                r = wpool.tile([P, NCH, H], bf16, tag="r")
                nc.scalar.activation(r[:], t[:],
                                     mybir.ActivationFunctionType.Relu)
                if 0 < ksplit < H:
                    lgp = wpool.tile([P, NCH], fp32, tag="lgp")
                    lgn = wpool.tile([P, NCH], fp32, tag="lgn")
                    nc.vector.tensor_reduce(lgp[:], r[:, :, :ksplit],
                                            axis=mybir.AxisListType.X,
                                            op=mybir.AluOpType.add)
                    nc.vector.tensor_reduce(lgn[:], r[:, :, ksplit:],
                                            axis=mybir.AxisListType.X,
                                            op=mybir.AluOpType.add)
                    lg = wpool.tile([P, NCH], fp32, tag="lg")
                    nc.vector.tensor_tensor(lg[:], lgp[:], lgn[:],
                                            op=mybir.AluOpType.subtract)
                else:
                    lg = wpool.tile([P, NCH], fp32, tag="lg")
                    nc.vector.tensor_reduce(lg[:], r[:],
                                            axis=mybir.AxisListType.X,
                                            op=mybir.AluOpType.add,
                                            negate=(ksplit == 0))

                nc.scalar.activation(
                    coll[:, (sg % 2) * NCH:(sg % 2) * NCH + NCH], lg[:],
                    mybir.ActivationFunctionType.Sigmoid, bias=b2rep[:])
                if sg % 2 == 1:
                    nc.sync.dma_start(out=out_d[g0 // SB], in_=coll[:])
                    if sg + 1 < nsup:
                        coll = wpool.tile([P, SB * 4], fp32, tag="coll",
                                          bufs=2)
    nc.compile()
    return nc


def _pack_core(row_loc, col_loc):
    """Sort by col, pack into 512-edge groups with <=128 distinct cols."""
    n = len(row_loc)
    order = np.argsort(col_loc, kind="stable")
    col_s = col_loc[order]
    ngroups = -(-n // GE) if n else 0

    isnew = np.ones(n, dtype=bool)
    if n > 1:
        isnew[1:] = col_s[1:] != col_s[:-1]
    isnew[::GE] = True
    chunk_id = np.arange(n) // GE
    distinct = np.bincount(chunk_id[isnew], minlength=max(ngroups, 1))

    if n == 0 or distinct.max() <= P:
        slot_run = np.cumsum(isnew) - 1
        base = np.concatenate([[0], np.cumsum(distinct)])[:-1]
        col_local_s = (slot_run - base[chunk_id]).astype(np.int64) if n else \
            np.zeros(0, np.int64)
        q_sorted = np.arange(n)
    else:
        col_local_s = np.empty(n, dtype=np.int64)
        q_sorted = np.empty(n, dtype=np.int64)
        g = 0
        cnt = 0
        slots = 0
        prev = -1
        bounds = []
        for i in range(n):
            c = col_s[i]
            new = c != prev
            if cnt == GE or (new and slots == P):
                bounds.append(i)
                g += 1
                cnt = 0
                slots = 0
                new = True
            if new:
                slots += 1
            col_local_s[i] = slots - 1
            q_sorted[i] = g * GE + cnt
            cnt += 1
            prev = c
        ngroups = g + 1
        isnew = np.ones(n, dtype=bool)
        isnew[1:] = col_s[1:] != col_s[:-1]
        for b in bounds:
            isnew[b] = True
        chunk_id = q_sorted // GE

    ecap_r = ngroups * GE
    col_local = np.zeros(ecap_r, dtype=np.int8)
    u_idx = np.zeros(ecap_r, dtype=np.int64)
    col_local[q_sorted] = col_local_s.astype(np.int8)
    u_idx[q_sorted] = row_loc[order]

    zd_rows = np.zeros(max(ngroups, 1) * P, dtype=np.int64)
    slot_pos = chunk_id[isnew] * P + col_local_s[isnew]
    zd_rows[slot_pos] = col_s[isnew]

    q_of_edge = np.empty(n, dtype=np.int64)
    q_of_edge[order] = q_sorted
    return q_of_edge, u_idx, col_local, zd_rows, ngroups


def _run(inputs, trace=False):
    global _last_results
    import ml_dtypes

    z_src = np.asarray(inputs["z_src"], dtype=np.float32)
    z_dst = np.asarray(inputs["z_dst"], dtype=np.float32)
    eli = np.asarray(inputs["edge_label_index"])
    row = np.ascontiguousarray(eli[0]).astype(np.int64)
    col = np.ascontiguousarray(eli[1]).astype(np.int64)
    W1 = np.asarray(inputs["W1"], dtype=np.float32)
    b1 = np.asarray(inputs["b1"], dtype=np.float32)
    W2 = np.asarray(inputs["W2"], dtype=np.float32)
    b2 = np.asarray(inputs["b2"], dtype=np.float32)

    ws = row // WIN_SRC
    wd = col // WIN_DST
    bucket = ws * 2 + wd
    perm = np.argsort(bucket, kind="stable")
    counts = np.bincount(bucket, minlength=N_CORES)
    starts = np.concatenate([[0], np.cumsum(counts)])

    packs = []
    for c in range(N_CORES):
        sel = perm[starts[c]:starts[c + 1]]
        r_loc = row[sel] - (c // 2) * WIN_SRC
        c_loc = col[sel] - (c % 2) * WIN_DST
        packs.append((sel, _pack_core(r_loc, c_loc)))

    ncap = max(p[1][4] for p in packs)
    ncap = SB * (-(-ncap // SB))
    ngath = ncap // SG
    ecap = ncap * GE

    # fold |w2| into W1a/W1b/b1; permute features so w2>0 comes first
    w2v = W2.reshape(-1).astype(np.float64)
    hperm = np.argsort(w2v <= 0, kind="stable")
    ksplit = int((w2v > 0).sum())
    aw2 = np.abs(w2v)[hperm].astype(np.float32)
    W1a = W1[:, :H][hperm] * aw2[:, None]
    W1b = W1[:, H:][hperm] * aw2[:, None]
    b1p = b1[hperm] * aw2

    key = (ncap, ksplit)
    if _cache.get("key") != key:
        _cache["nc"] = _build_program(ncap, ksplit)
        _cache["key"] = key
    nc = _cache["nc"]

    w1aT = np.ascontiguousarray(W1a.T)
    w1bT = np.ascontiguousarray(W1b.T)
    b1rep = np.ascontiguousarray(np.tile(b1p[None, :], (P, 1)))
    b2rep = np.full((P, 1), float(b2.reshape(-1)[0]), dtype=np.float32)

    in_maps = []
    for c in range(N_CORES):
        sel, (q_of_edge, u_idx, col_local, zd_rows, ngroups) = packs[c]
        zsw = z_src[(c // 2) * WIN_SRC:(c // 2 + 1) * WIN_SRC]
        zdw = z_dst[(c % 2) * WIN_DST:(c % 2 + 1) * WIN_DST]

        zsT = np.zeros((UTAB_ROWS, H), dtype=ml_dtypes.bfloat16)
        zsT[:WIN_SRC] = zsw.astype(ml_dtypes.bfloat16)
        zsT = np.ascontiguousarray(zsT.T)

        zdr = np.zeros(ncap * P, dtype=np.int64)
        zdr[:len(zd_rows)] = zd_rows
        zdT = np.ascontiguousarray(zdw[zdr].astype(ml_dtypes.bfloat16).T)

        ui = np.zeros(ngath * BG, dtype=np.int64)
        ui[:len(u_idx)] = _uslot(u_idx)
        ix = _wrap_idx_all(ui, ngath)

        cl = np.zeros(ecap, dtype=np.int64)
        cl[:len(col_local)] = col_local
        Sh = np.zeros((P, ecap), dtype=ml_dtypes.bfloat16)
        Sh[cl, np.arange(ecap)] = 1

        in_maps.append({
            "zsT": zsT, "zdT": zdT, "ix": ix, "S": Sh,
            "w1aT": w1aT, "w1bT": w1bT, "b1rep": b1rep,
            "b2rep": b2rep,
        })

    try:
        res = run_bass_kernel_spmd(nc, in_maps, core_ids=list(range(N_CORES)),
                                   trace=trace)
    except ImportError:
        os.environ.pop("BASS_TRACE", None)
        res = run_bass_kernel_spmd(nc, in_maps, core_ids=list(range(N_CORES)),
                                   trace=False)
    _last_results = res

    out = np.empty(E, dtype=np.float32)
    for c in range(N_CORES):
        sel, (q_of_edge, _, _, _, _) = packs[c]
        dev = res.results[c]["out"].reshape(-1)
        q = q_of_edge
        g = q // GE
        within = q % GE
        p_ = within % P
        ch = within // P
        pos = (g // SB) * (P * SB * 4) + p_ * (SB * 4) + (g % SB) * 4 + ch
        out[sel] = dev[pos]
    return out


def kernel(**inputs):
    return _run(inputs, trace=bool(os.environ.get("BASS_TRACE")))
